# revision 27
# baseline (speedup 1.0000x reference)
"""Trainium2 Bass kernel: single-head attention (B=4, N=2048, D=1024).

Sharding: 8 cores = (batch b, query-half h). Each core computes attention for
its 1024 queries against all 2048 keys of its batch. K/V projections are
deduplicated: each core projects K/V only for its own 1024 keys, then core
pairs (2b, 2b+1) AllGather the halves (global key order).

Algebraic folds (host side):
  - W2 = Wv @ Wo collapses the V projection and the output projection into a
    single projection: out = softmax(S) @ (x @ W2) + bo_eff. This removes the
    entire output-projection matmul phase.
  - bo_eff = bo + bv @ Wo is a constant row added to the assembled output
    on the host (softmax rows sum to 1), at zero device cost.
  - bk cancels in softmax and is dropped; bq is added on-device.

Precision: the scores matmul runs in fp8-e4m3 with perf_mode=DoubleRow
(2 contraction rows per PE pass): QT/KT are written as e4m3 (values ~N(0,
0.64) fit comfortably), and the 1/sqrt(N) softmax scale is applied inside
the Exp activation instead of being folded into Wq (which would push Q
subnormal in fp8). Projections and the AV matmul stay bf16 (fp8 there
breaks the 2e-2 tolerance). K is exchanged between pair cores as fp8,
halving collective bytes. The softmax row-sum vector s is handled in bf16
(0.1% rms, negligible vs the fp8 scores error).

Startup latency: wk and x are loaded as per-db chunk tiles with
first-needed-first interleaved triggers on the sync queue, so the first
matmul waits on 384 KB, not 3 MB. Deferred loads (x tile 1, w2, wq) issue
from the scalar queue; mask (full-width rows) after the wk/x0 interleave on
sync. PSUM->SBUF copies in phase A are pinned to the vector engine so the
scalar queue stays clear for its DMA triggers.

Layouts avoid all on-device transposes:
  - QT, KTh produced in [e, n] layout     (lhsT = W as stored, rhs = x^T)
  - V2h produced in [key, e] layout       (lhsT = x^T block,  rhs = W2)
  - scores computed transposed ST[key, q] (lhsT = KT pair-block, rhs = QT,
                                           DoubleRow over e-block pairs)
  - out[e, q] = V2-block^T @ UT           (lhsT = V2 block, rhs = UT)
Host transposes x on the way in and outT on the way out.

Softmax: raw scores are O(+-80) in f32 PSUM; exp applies scale=1/sqrt(N)
so exp() without max-subtraction is safe. Mask applied additively (-1e9)
before the scaled exp. Row sums via ones-vector matmul over UT's partition
axis; normalization via K=1 broadcast matmul of s and a DVE
reciprocal+multiply.
"""

import sys

if "/opt/trn_rl_repo" not in sys.path:
    sys.path.insert(0, "/opt/trn_rl_repo")

import numpy as np
import ml_dtypes

B, N, D = 4, 2048, 1024
P = 128
NQ = N // 2          # queries (and locally-projected keys) per core
DB = D // P          # 8   d/e blocks of 128
SB = DB // 2         # 4   fp8 DoubleRow superblocks (pairs of e-blocks)
KB = N // P          # 16  key blocks of 128
KBH = NQ // P        # 8   key blocks per half
FT = 512             # matmul free-dim tile
NTH = NQ // FT       # 2   n tiles over own half
QT_TILES = NQ // FT  # 2   q tiles
ET = D // FT         # 2   e tiles
INV_SQRT_N = 1.0 / float(np.sqrt(np.float32(N)))

BF16 = ml_dtypes.bfloat16
MASK_NEG = -1.0e9

_cached = None


def _build_program():
    import concourse.bacc as bacc
    import concourse.mybir as mybir
    import concourse.tile as tile

    f32 = mybir.dt.float32
    bf16 = mybir.dt.bfloat16
    fp8 = mybir.dt.float8e4
    AF = mybir.ActivationFunctionType
    ALU = mybir.AluOpType
    DR = mybir.MatmulPerfMode.DoubleRow
    PAIRS = [[0, 1], [2, 3], [4, 5], [6, 7]]

    nc = bacc.Bacc("TRN2", target_bir_lowering=False, debug=False, num_devices=8)

    xTh = nc.dram_tensor("xTh", [D, NQ], bf16, kind="ExternalInput").ap()
    maskadd = nc.dram_tensor("maskadd", [N, NQ], bf16, kind="ExternalInput").ap()
    wq = nc.dram_tensor("wq", [D, D], bf16, kind="ExternalInput").ap()
    wk = nc.dram_tensor("wk", [D, D], bf16, kind="ExternalInput").ap()
    w2 = nc.dram_tensor("w2", [D, D], bf16, kind="ExternalInput").ap()
    bq = nc.dram_tensor("bq", [D], f32, kind="ExternalInput").ap()
    outT = nc.dram_tensor("outT", [D, NQ], f32, kind="ExternalOutput").ap()

    xTh_r = xTh.rearrange("(db p) n -> p db n", p=P)
    wq_r = wq.rearrange("(db p) e -> p db e", p=P)
    wk_r = wk.rearrange("(db p) e -> p db e", p=P)
    w2_r = w2.rearrange("(db p) e -> p db e", p=P)
    bq_r = bq.rearrange("(eb p) -> p eb", p=P)

    with tile.TileContext(nc) as tc:
        with (
            tc.tile_pool(name="const", bufs=1) as const,
            tc.tile_pool(name="persist", bufs=1) as persist,
            tc.tile_pool(name="dram", bufs=1, space="DRAM") as dram,
        ):
            bq_t = const.tile([P, DB], f32)
            ones_k = const.tile([P, 1], bf16)
            nc.vector.memset(ones_k, 1.0)
            ones_b = const.tile([1, P], bf16)
            nc.vector.memset(ones_b, 1.0)

            QT = persist.tile([P, DB, NQ], fp8)
            KT = persist.tile([P, DB, N], fp8)
            V = persist.tile([P, KB, D], bf16)
            maskfull = persist.tile([P, KB, NQ], bf16)

            # K exchange split into two column-chunk AllGathers (they
            # pipeline on the CC stream and finish early); V as ONE
            # collective — measured: collective latency is ~15-25us nearly
            # independent of size, and a split V serializes +26us per chunk.
            ktha_d = dram.tile([D, FT], fp8)
            kthb_d = dram.tile([D, FT], fp8)
            ktga_d = dram.tile([2, D, FT], fp8)
            ktgb_d = dram.tile([2, D, FT], fp8)
            vh_d = dram.tile([NQ, D], bf16)
            vg_d = dram.tile([2, NQ, D], bf16)

            # ---- Phase A: project K half, V2 half (collectives), then Q ----
            with (
                tc.tile_pool(name="wpool", bufs=1) as wpool,
                tc.tile_pool(name="xpool", bufs=1) as xpool,
            ):
                # wk/x0 as per-db chunk tiles, triggers interleaved in
                # first-needed order on the sync queue: the first matmul
                # depends on one wk chunk + one x chunk only.
                # wk0 is itself split in half: the first matmul group
                # (eb 0-3) needs only cols 0-511 of wk0.
                wk0h = [wpool.tile([P, FT], bf16, name=f"wk0h{h}")
                        for h in range(2)]
                wk_ts = [None] + [wpool.tile([P, D], bf16, name=f"wk{db}")
                                  for db in range(1, DB)]
                x0_ts = [xpool.tile([P, FT], bf16, name=f"x0_{db}")
                         for db in range(DB)]
                x1_ts = [xpool.tile([P, FT], bf16, name=f"x1_{db}")
                         for db in range(DB)]
                # First three db pairs from sync, the rest from the head
                # of the scalar queue: two queues issue triggers in
                # parallel (the ~0.7us per-trigger engine cost is the
                # startup bottleneck, not transfer bandwidth).
                nc.sync.dma_start(out=wk0h[0], in_=wk_r[:, 0, 0:FT])
                nc.sync.dma_start(out=x0_ts[0], in_=xTh_r[:, 0, 0:FT])
                nc.sync.dma_start(out=wk0h[1], in_=wk_r[:, 0, FT:D])
                for db in range(1, 3):
                    nc.sync.dma_start(out=wk_ts[db], in_=wk_r[:, db, :])
                    nc.sync.dma_start(
                        out=x0_ts[db], in_=xTh_r[:, db, 0:FT]
                    )
                for db in range(3, DB):
                    nc.scalar.dma_start(out=wk_ts[db], in_=wk_r[:, db, :])
                    nc.scalar.dma_start(
                        out=x0_ts[db], in_=xTh_r[:, db, 0:FT]
                    )
                # Deferred loads split across BOTH queues to bound trigger
                # issue-serialization (~0.65us per trigger on-engine):
                # x1/w2 continue on sync (idle after the startup pairs);
                # bq/boe/wq/mask on scalar after its startup share.
                for db in range(DB):
                    nc.sync.dma_start(
                        out=x1_ts[db], in_=xTh_r[:, db, FT:NQ]
                    )
                w2_t = wpool.tile([P, DB, D], bf16)
                for db in range(DB):
                    nc.sync.dma_start(out=w2_t[:, db, :], in_=w2_r[:, db, :])
                nc.scalar.dma_start(out=bq_t, in_=bq_r)
                wq_t = wpool.tile([P, DB, D], bf16)
                for db in range(DB):
                    nc.scalar.dma_start(out=wq_t[:, db, :], in_=wq_r[:, db, :])
                # Whole mask loads last (needed only by phase B): full-width
                # rows (2KB contiguous runs, 16 triggers) at the scalar
                # queue's tail so its transfers never crowd out x1/w2/wq.
                for kb in range(KB):
                    nc.scalar.dma_start(
                        out=maskfull[:, kb, :],
                        in_=maskadd[kb * P : (kb + 1) * P, :],
                    )

                # K projection (own half), written as fp8 for the DoubleRow
                # scores matmul. First n-tile is db-outer so the first matmul
                # only needs one wk chunk + one x chunk (8 concurrent PSUM
                # groups); the rest uses a 4-buf pool. PSUM copies pinned to
                # vector (scalar queue is busy with DMA triggers).
                kthpool_ctx = tc.tile_pool(name="kthpool", bufs=1)
                kthpool = kthpool_ctx.__enter__()
                kth_t = kthpool.tile([P, DB, NQ], fp8)
                # psA (4 banks) coexists with psK0 (4 banks): K tile 1 never
                # waits on tile 0's PSUM copies.
                psA_ctx = tc.tile_pool(name="psA", bufs=4, space="PSUM")
                psA = psA_ctx.__enter__()
                # Two eb-groups of 4: only 4 PSUM banks held at a time, so
                # group 0's copies (and the bank release the next pool waits
                # on) overlap group 1's matmuls.
                def wk_sl(db, eb):
                    if db == 0:
                        h, ebh = divmod(eb, 4)
                        return wk0h[h][:, ebh * P : (ebh + 1) * P]
                    return wk_ts[db][:, eb * P : (eb + 1) * P]

                with tc.tile_pool(name="psK0", bufs=1, space="PSUM") as psK:
                    # All 8 eb accumulators live at once (4 psK banks + 4
                    # borrowed from psA): one full pass over db consumes
                    # wk/x chunks at the rate the startup DMAs supply them.
                    pss = [
                        psK.tile([P, FT], f32, name=f"psk0_{eb}")
                        for eb in range(4)
                    ] + [
                        psA.tile([P, FT], f32, tag="ps", name=f"psk1_{eb}")
                        for eb in range(4)
                    ]
                    for db in range(DB):
                        for eb in range(DB):
                            nc.tensor.matmul(
                                pss[eb],
                                lhsT=wk_sl(db, eb),
                                rhs=x0_ts[db],
                                start=(db == 0),
                                stop=(db == DB - 1),
                            )
                    with nc.allow_low_precision(reason="fp8 K for DoubleRow"):
                        for eb in range(DB):
                            nc.vector.tensor_copy(
                                out=kth_t[:, eb, 0:FT], in_=pss[eb]
                            )
                ka_r = ktha_d.rearrange("(db p) n -> p db n", p=P)
                nc.gpsimd.dma_start(out=ka_r[:, 0:4, :], in_=kth_t[:, 0:4, 0:FT])
                nc.gpsimd.dma_start(out=ka_r[:, 4:8, :], in_=kth_t[:, 4:8, 0:FT])
                nc.gpsimd.collective_compute(
                    "AllGather",
                    ALU.bypass,
                    replica_groups=PAIRS,
                    ins=[ktha_d[:, :]],
                    outs=[ktga_d[:, :, :]],
                )

                for eb in range(DB):
                    ps = psA.tile([P, FT], f32, tag="ps")
                    for db in range(DB):
                        nc.tensor.matmul(
                            ps,
                            lhsT=wk_sl(db, eb),
                            rhs=x1_ts[db],
                            start=(db == 0),
                            stop=(db == DB - 1),
                        )
                    with nc.allow_low_precision(reason="fp8 K for DoubleRow"):
                        nc.vector.tensor_copy(out=kth_t[:, eb, FT:NQ], in_=ps)
                kb_r = kthb_d.rearrange("(db p) n -> p db n", p=P)
                nc.gpsimd.dma_start(out=kb_r[:, 0:4, :], in_=kth_t[:, 0:4, FT:NQ])
                nc.gpsimd.dma_start(out=kb_r[:, 4:8, :], in_=kth_t[:, 4:8, FT:NQ])
                nc.gpsimd.collective_compute(
                    "AllGather",
                    ALU.bypass,
                    replica_groups=PAIRS,
                    ins=[kthb_d[:, :]],
                    outs=[ktgb_d[:, :, :]],
                )

                # V2 projection runs EARLY (right after the first K chunk) so its
                # larger collective finishes long before the AV phase
                # needs V; K tile 1 + its collective follow.
                # kthpool stays open: freeing it here would let vh_t reuse
                # kth_t's SBUF, making every V2 copy wait (WAR) on the K
                # staging DMAs.
                vhpool_ctx = tc.tile_pool(name="vhpool", bufs=1)
                vhpool = vhpool_ctx.__enter__()
                vh_t = vhpool.tile([P, KBH, D], bf16)
                x_ts = [x0_ts, x1_ts]
                for ks in range(KBH):
                    nt, kso = divmod(ks, FT // P)
                    for et in range(ET):
                        esl = slice(et * FT, (et + 1) * FT)
                        ps = psA.tile([P, FT], f32, tag="ps")
                        for db in range(DB):
                            nc.tensor.matmul(
                                ps,
                                lhsT=x_ts[nt][db][:, kso * P : (kso + 1) * P],
                                rhs=w2_t[:, db, esl],
                                start=(db == 0),
                                stop=(db == DB - 1),
                            )
                        nc.vector.tensor_copy(out=vh_t[:, ks, esl], in_=ps)
                vh_dr = vh_d.rearrange("(kb p) e -> p kb e", p=P)
                nc.gpsimd.dma_start(out=vh_dr[:, 0:4, :], in_=vh_t[:, 0:4, :])
                nc.gpsimd.dma_start(out=vh_dr[:, 4:8, :], in_=vh_t[:, 4:8, :])
                nc.gpsimd.collective_compute(
                    "AllGather",
                    ALU.bypass,
                    replica_groups=PAIRS,
                    ins=[vh_d[:, :]],
                    outs=[vg_d[:, :, :]],
                )


                # Loads issue after all collectives so the in-order gpsimd
                # queue never stalls a collective trigger behind a load.
                # Merged 3D-AP triggers (one per gather chunk).
                for nt, kg in enumerate([ktga_d, ktgb_d]):
                    for g in range(2):
                        kg_gr = kg[g].rearrange("(db p) n -> p db n", p=P)
                        off = g * NQ + nt * FT
                        nc.sync.dma_start(
                            out=KT[:, :, off : off + FT], in_=kg_gr[:, :, :]
                        )
                for g in range(2):
                    vg_gr = vg_d[g].rearrange("(kb p) e -> p kb e", p=P)
                    nc.gpsimd.dma_start(
                        out=V[:, g * KBH : (g + 1) * KBH, :],
                        in_=vg_gr[:, :, :],
                    )

                # Q projection (overlaps the collectives), written as fp8
                with nc.allow_low_precision(reason="fp8 Q for DoubleRow"):
                    for nt in range(NTH):
                        nsl = slice(nt * FT, (nt + 1) * FT)
                        for eb in range(DB):
                            ps = psA.tile([P, FT], f32, tag="ps")
                            for db in range(DB):
                                nc.tensor.matmul(
                                    ps,
                                    lhsT=wq_t[:, db, eb * P : (eb + 1) * P],
                                    rhs=x_ts[nt][db],
                                    start=(db == 0),
                                    stop=(db == DB - 1),
                                )
                            nc.scalar.activation(
                                out=QT[:, eb, nsl],
                                in_=ps,
                                func=AF.Identity,
                                bias=bq_t[:, eb : eb + 1],
                                scale=1.0,
                            )
                psA_ctx.__exit__(None, None, None)
                vhpool_ctx.__exit__(None, None, None)
                kthpool_ctx.__exit__(None, None, None)

            # ---- Phase B: scores (fp8 DoubleRow), softmax, AV, output ----
            with tc.tile_pool(name="persist2", bufs=1) as persist2:
                UT = persist2.tile([P, KB, NQ], bf16)
                sbcs = [persist2.tile([P, FT], f32, name=f"sbc{qt}")
                        for qt in range(QT_TILES)]
                s_bf = [persist2.tile([1, FT], bf16, name=f"sbf{qt}")
                        for qt in range(QT_TILES)]

                with (
                    tc.tile_pool(name="scr", bufs=3) as scr,
                    tc.tile_pool(name="psST", bufs=4, space="PSUM") as psST,
                    tc.tile_pool(name="psAV", bufs=3, space="PSUM") as psAV,
                    tc.tile_pool(name="psS", bufs=1, space="PSUM") as psS,
                    tc.tile_pool(name="ost", bufs=3) as ostp,
                ):
                    # Consume keys in K-gather arrival order:
                    # gather 0 -> kb {0-3, 8-11}, gather 1 -> kb {4-7, 12-15}
                    ST_KB_HALVES = [[0, 1, 2, 3, 8, 9, 10, 11],
                                    [4, 5, 6, 7, 12, 13, 14, 15]]

                    def st_tiles(qt, half):
                        qsl = slice(qt * FT, (qt + 1) * FT)
                        for kb in ST_KB_HALVES[half]:
                            ps = psST.tile([P, FT], f32, tag="st", name=f"ps_st{qt}_{kb}")
                            for s in range(SB):
                                nc.tensor.matmul(
                                    ps,
                                    lhsT=KT[:, 2 * s : 2 * s + 2,
                                            kb * P : (kb + 1) * P],
                                    rhs=QT[:, 2 * s : 2 * s + 2, qsl],
                                    start=(s == 0),
                                    stop=(s == SB - 1),
                                    perf_mode=DR,
                                )
                            sc = scr.tile([P, FT], f32, tag="sc", name=f"sc{qt}_{kb}")
                            nc.vector.tensor_tensor(
                                sc, ps, maskfull[:, kb, qsl], op=ALU.add
                            )
                            nc.scalar.activation(
                                out=UT[:, kb, qsl], in_=sc, func=AF.Exp,
                                scale=INV_SQRT_N,
                            )

                    def sums(qt):
                        qsl = slice(qt * FT, (qt + 1) * FT)
                        pss = psS.tile([1, FT], f32, tag="pss", name=f"pss{qt}")
                        for kb in range(KB):
                            nc.tensor.matmul(
                                pss,
                                lhsT=ones_k,
                                rhs=UT[:, kb, qsl],
                                start=(kb == 0),
                                stop=(kb == KB - 1),
                            )
                        with nc.allow_low_precision(
                            reason="softmax sums in bf16; 0.1% rms"
                        ):
                            nc.vector.tensor_copy(out=s_bf[qt], in_=pss)

                    def av_out(qt):
                        qsl = slice(qt * FT, (qt + 1) * FT)
                        psb = psAV.tile([P, FT], f32, tag="av", name=f"ps_b{qt}")
                        # bf16 K=1 broadcast of s across partitions; DVE
                        # reciprocal after, 128 partitions wide.
                        nc.tensor.matmul(
                            psb, lhsT=ones_b, rhs=s_bf[qt], start=True, stop=True
                        )
                        nc.vector.reciprocal(out=sbcs[qt], in_=psb)
                        for eb in range(DB):
                            ps = psAV.tile([P, FT], f32, tag="av", name=f"ps_av{qt}_{eb}")
                            for kb in range(KB):
                                nc.tensor.matmul(
                                    ps,
                                    lhsT=V[:, kb, eb * P : (eb + 1) * P],
                                    rhs=UT[:, kb, qsl],
                                    start=(kb == 0),
                                    stop=(kb == KB - 1),
                                )
                            if qt == QT_TILES - 1 and eb == DB - 1:
                                # Final tile: halve the epilogue chain so the
                                # first out-DMA overlaps the second multiply.
                                HF = FT // 2
                                for h in range(2):
                                    hs = slice(h * HF, (h + 1) * HF)
                                    hq = slice(qt * FT + h * HF,
                                               qt * FT + (h + 1) * HF)
                                    ot = ostp.tile([P, HF], f32, tag="oth")
                                    nc.vector.tensor_mul(
                                        ot, ps[:, hs], sbcs[qt][:, hs]
                                    )
                                    nc.sync.dma_start(
                                        out=outT[eb * P : (eb + 1) * P, hq],
                                        in_=ot,
                                    )
                            else:
                                ot = ostp.tile([P, FT], f32, tag="ot")
                                nc.vector.tensor_mul(ot, ps, sbcs[qt])
                                nc.sync.dma_start(
                                    out=outT[eb * P : (eb + 1) * P, qsl],
                                    in_=ot,
                                )

                    # Order chosen so every PE wait is covered by prior PE
                    # work: both q-tiles' gather-0 keys run before any
                    # gather-1 key is needed.
                    st_tiles(0, 0)
                    st_tiles(1, 0)
                    st_tiles(0, 1)
                    sums(0)
                    st_tiles(1, 1)
                    sums(1)
                    av_out(0)
                    av_out(1)

    nc.compile()
    return nc


def _get_program():
    global _cached
    if _cached is None:
        _cached = _build_program()
    return _cached


def make_in_maps(x, mask, Wq, bq, Wk, bk, Wv, bv, Wo, bo):
    """Host-side preprocessing: per-core input dicts."""
    wq_b = Wq.astype(BF16)
    wk_b = Wk.astype(BF16)
    w2_b = (Wv.astype(np.float64) @ Wo.astype(np.float64)).astype(BF16)
    bq_f = bq.astype(np.float32)

    in_maps = []
    for c in range(8):
        b, h = divmod(c, 2)
        qs = slice(h * NQ, (h + 1) * NQ)
        xTh_c = np.ascontiguousarray(x[b, qs].T).astype(BF16)  # [D, NQ]
        madd = np.where(
            mask[b, qs, :].T, np.float32(MASK_NEG), np.float32(0.0)
        ).astype(BF16)  # [N, NQ], global key order
        in_maps.append(
            {
                "xTh": xTh_c,
                "maskadd": np.ascontiguousarray(madd),
                "wq": wq_b,
                "wk": wk_b,
                "w2": w2_b,
                "bq": bq_f,
            }
        )
    return in_maps


def assemble(results):
    out = np.empty((B, N, D), dtype=np.float32)
    for c in range(8):
        b, h = divmod(c, 2)
        out[b, h * NQ : (h + 1) * NQ, :] = results[c]["outT"].T
    return out


def kernel(x, mask, Wq, bq, Wk, bk, Wv, bv, Wo, bo):
    from concourse.bass_utils import run_bass_kernel_spmd

    nc = _get_program()
    in_maps = make_in_maps(x, mask, Wq, bq, Wk, bk, Wv, bv, Wo, bo)
    res = run_bass_kernel_spmd(nc, in_maps, list(range(8)))
    out = assemble(res.results)
    # Attention rows sum to 1, so bo_eff = bo + bv @ Wo is a constant row
    # of the output; adding it host-side costs no device time.
    bo_eff = (
        bo.astype(np.float64) + bv.astype(np.float64) @ Wo.astype(np.float64)
    ).astype(np.float32)
    out += bo_eff
    return out


# revision 28
# speedup vs baseline: 1.1915x; 1.1915x over previous
"""Trainium2 Bass kernel: single-head attention (B=4, N=2048, D=1024).

Sharding: 8 cores = (batch b, query-half h). Each core computes attention for
its 1024 queries against all 2048 keys of its batch. K/V projections are
deduplicated: each core projects K/V only for its own 1024 keys, then core
pairs (2b, 2b+1) AllGather the halves (global key order).

Algebraic folds (host side):
  - W2 = Wv @ Wo collapses the V projection and the output projection into a
    single projection: out = softmax(S) @ (x @ W2) + bo_eff. This removes the
    entire output-projection matmul phase.
  - bo_eff = bo + bv @ Wo is a constant row added to the assembled output
    on the host (softmax rows sum to 1), at zero device cost.
  - bk cancels in softmax and is dropped; bq is added on-device.

Precision: the scores matmul runs in fp8-e4m3 with perf_mode=DoubleRow
(2 contraction rows per PE pass): QT/KT are written as e4m3 (values ~N(0,
0.64) fit comfortably), and the 1/sqrt(N) softmax scale is applied inside
the Exp activation instead of being folded into Wq (which would push Q
subnormal in fp8). Projections and the AV matmul stay bf16 (fp8 there
breaks the 2e-2 tolerance). K is exchanged between pair cores as fp8,
halving collective bytes. The softmax row-sum vector s is handled in bf16
(0.1% rms, negligible vs the fp8 scores error).

Startup latency: wk and x are loaded as per-db chunk tiles with
first-needed-first interleaved triggers on the sync queue, so the first
matmul waits on 384 KB, not 3 MB. Deferred loads (x tile 1, w2, wq) issue
from the scalar queue; mask (full-width rows) after the wk/x0 interleave on
sync. PSUM->SBUF copies in phase A are pinned to the vector engine so the
scalar queue stays clear for its DMA triggers.

Layouts avoid all on-device transposes:
  - QT, KTh produced in [e, n] layout     (lhsT = W as stored, rhs = x^T)
  - V2h produced in [key, e] layout       (lhsT = x^T block,  rhs = W2)
  - scores computed transposed ST[key, q] (lhsT = KT pair-block, rhs = QT,
                                           DoubleRow over e-block pairs)
  - out[e, q] = V2-block^T @ UT           (lhsT = V2 block, rhs = UT)
Host transposes x on the way in and outT on the way out.

Softmax: raw scores are O(+-80) in f32 PSUM; exp applies scale=1/sqrt(N)
so exp() without max-subtraction is safe. Mask applied additively (-1e9)
before the scaled exp. Row sums via ones-vector matmul over UT's partition
axis; normalization via K=1 broadcast matmul of s and a DVE
reciprocal+multiply.
"""

import sys

if "/opt/trn_rl_repo" not in sys.path:
    sys.path.insert(0, "/opt/trn_rl_repo")

import numpy as np
import ml_dtypes

B, N, D = 4, 2048, 1024
P = 128
NQ = N // 2          # queries (and locally-projected keys) per core
DB = D // P          # 8   d/e blocks of 128
SB = DB // 2         # 4   fp8 DoubleRow superblocks (pairs of e-blocks)
KB = N // P          # 16  key blocks of 128
KBH = NQ // P        # 8   key blocks per half
FT = 512             # matmul free-dim tile
NTH = NQ // FT       # 2   n tiles over own half
QT_TILES = NQ // FT  # 2   q tiles
ET = D // FT         # 2   e tiles
INV_SQRT_N = 1.0 / float(np.sqrt(np.float32(N)))

BF16 = ml_dtypes.bfloat16
MASK_NEG = -1.0e9

_cached = None


def _build_program():
    import concourse.bacc as bacc
    import concourse.mybir as mybir
    import concourse.tile as tile

    f32 = mybir.dt.float32
    bf16 = mybir.dt.bfloat16
    fp8 = mybir.dt.float8e4
    AF = mybir.ActivationFunctionType
    ALU = mybir.AluOpType
    DR = mybir.MatmulPerfMode.DoubleRow
    PAIRS = [[0, 1], [2, 3], [4, 5], [6, 7]]

    nc = bacc.Bacc("TRN2", target_bir_lowering=False, debug=False, num_devices=8)

    xTh = nc.dram_tensor("xTh", [D, NQ], bf16, kind="ExternalInput").ap()
    maskadd = nc.dram_tensor("maskadd", [N, NQ], bf16, kind="ExternalInput").ap()
    wq = nc.dram_tensor("wq", [D, D], bf16, kind="ExternalInput").ap()
    wk = nc.dram_tensor("wk", [D, D], bf16, kind="ExternalInput").ap()
    w2 = nc.dram_tensor("w2", [D, D], bf16, kind="ExternalInput").ap()
    bq = nc.dram_tensor("bq", [D], f32, kind="ExternalInput").ap()
    outT = nc.dram_tensor("outT", [D, NQ], f32, kind="ExternalOutput").ap()

    xTh_r = xTh.rearrange("(db p) n -> p db n", p=P)
    wq_r = wq.rearrange("(db p) e -> p db e", p=P)
    wk_r = wk.rearrange("(db p) e -> p db e", p=P)
    w2_r = w2.rearrange("(db p) e -> p db e", p=P)
    bq_r = bq.rearrange("(eb p) -> p eb", p=P)

    with tile.TileContext(nc) as tc:
        with (
            tc.tile_pool(name="const", bufs=1) as const,
            tc.tile_pool(name="persist", bufs=1) as persist,
            tc.tile_pool(name="dram", bufs=1, space="DRAM") as dram,
        ):
            bq_t = const.tile([P, DB], f32)
            ones_k = const.tile([P, 1], bf16)
            nc.vector.memset(ones_k, 1.0)
            ones_b = const.tile([1, P], bf16)
            nc.vector.memset(ones_b, 1.0)

            QT = persist.tile([P, DB, NQ], fp8)
            KT = persist.tile([P, DB, N], fp8)
            V = persist.tile([P, KB, D], bf16)
            maskfull = persist.tile([P, KB, NQ], bf16)

            # K exchange split into two column-chunk AllGathers (they
            # pipeline on the CC stream and finish early); V as ONE
            # collective — measured: collective latency is ~15-25us nearly
            # independent of size, and a split V serializes +26us per chunk.
            ktha_d = dram.tile([D, FT], fp8)
            kthb_d = dram.tile([D, FT], fp8)
            ktga_d = dram.tile([2, D, FT], fp8)
            ktgb_d = dram.tile([2, D, FT], fp8)
            vh_d = dram.tile([NQ, D], bf16)
            vg_d = dram.tile([2, NQ, D], bf16)

            # ---- Phase A: project K half, V2 half (collectives), then Q ----
            with (
                tc.tile_pool(name="wpool", bufs=1) as wpool,
                tc.tile_pool(name="xpool", bufs=1) as xpool,
            ):
                # wk/x0 as per-db chunk tiles, triggers interleaved in
                # first-needed order on the sync queue: the first matmul
                # depends on one wk chunk + one x chunk only.
                # wk0 is itself split in half: the first matmul group
                # (eb 0-3) needs only cols 0-511 of wk0.
                wk0h = [wpool.tile([P, FT], bf16, name=f"wk0h{h}")
                        for h in range(2)]
                wk_ts = [None] + [wpool.tile([P, D], bf16, name=f"wk{db}")
                                  for db in range(1, DB)]
                x0_ts = [xpool.tile([P, FT], bf16, name=f"x0_{db}")
                         for db in range(DB)]
                x1_ts = [xpool.tile([P, FT], bf16, name=f"x1_{db}")
                         for db in range(DB)]
                # First three db pairs from sync, the rest from the head
                # of the scalar queue: two queues issue triggers in
                # parallel (the ~0.7us per-trigger engine cost is the
                # startup bottleneck, not transfer bandwidth).
                nc.sync.dma_start(out=wk0h[0], in_=wk_r[:, 0, 0:FT])
                nc.sync.dma_start(out=x0_ts[0], in_=xTh_r[:, 0, 0:FT])
                nc.sync.dma_start(out=wk0h[1], in_=wk_r[:, 0, FT:D])
                for db in range(1, 3):
                    nc.sync.dma_start(out=wk_ts[db], in_=wk_r[:, db, :])
                    nc.sync.dma_start(
                        out=x0_ts[db], in_=xTh_r[:, db, 0:FT]
                    )
                for db in range(3, DB):
                    nc.scalar.dma_start(out=wk_ts[db], in_=wk_r[:, db, :])
                    nc.scalar.dma_start(
                        out=x0_ts[db], in_=xTh_r[:, db, 0:FT]
                    )
                # Deferred loads split across BOTH queues to bound trigger
                # issue-serialization (~0.65us per trigger on-engine):
                # x1/w2 continue on sync (idle after the startup pairs);
                # bq/boe/wq/mask on scalar after its startup share.
                for db in range(DB):
                    nc.sync.dma_start(
                        out=x1_ts[db], in_=xTh_r[:, db, FT:NQ]
                    )
                w2_t = wpool.tile([P, DB, D], bf16)
                for db in range(DB):
                    nc.sync.dma_start(out=w2_t[:, db, :], in_=w2_r[:, db, :])
                nc.scalar.dma_start(out=bq_t, in_=bq_r)
                wq_t = wpool.tile([P, DB, D], bf16)
                for db in range(DB):
                    nc.scalar.dma_start(out=wq_t[:, db, :], in_=wq_r[:, db, :])
                # Whole mask loads last (needed only by phase B): full-width
                # rows (2KB contiguous runs, 16 triggers) at the scalar
                # queue's tail so its transfers never crowd out x1/w2/wq.
                for kb in range(KB):
                    nc.scalar.dma_start(
                        out=maskfull[:, kb, :],
                        in_=maskadd[kb * P : (kb + 1) * P, :],
                    )

                # K projection (own half), written as fp8 for the DoubleRow
                # scores matmul. First n-tile is db-outer so the first matmul
                # only needs one wk chunk + one x chunk (8 concurrent PSUM
                # groups); the rest uses a 4-buf pool. PSUM copies pinned to
                # vector (scalar queue is busy with DMA triggers).
                kthpool_ctx = tc.tile_pool(name="kthpool", bufs=1)
                kthpool = kthpool_ctx.__enter__()
                kth_t = kthpool.tile([P, DB, NQ], fp8)
                # psA (4 banks) coexists with psK0 (4 banks): K tile 1 never
                # waits on tile 0's PSUM copies.
                psA_ctx = tc.tile_pool(name="psA", bufs=4, space="PSUM")
                psA = psA_ctx.__enter__()
                # Two eb-groups of 4: only 4 PSUM banks held at a time, so
                # group 0's copies (and the bank release the next pool waits
                # on) overlap group 1's matmuls.
                def wk_sl(db, eb):
                    if db == 0:
                        h, ebh = divmod(eb, 4)
                        return wk0h[h][:, ebh * P : (ebh + 1) * P]
                    return wk_ts[db][:, eb * P : (eb + 1) * P]

                with tc.tile_pool(name="psK0", bufs=1, space="PSUM") as psK:
                    pss = [
                        psK.tile([P, FT], f32, name=f"psk0_{eb}")
                        for eb in range(4)
                    ]
                    for g in range(2):
                        ebs = range(4 * g, 4 * g + 4)
                        for db in range(DB):
                            for eb in ebs:
                                nc.tensor.matmul(
                                    pss[eb % 4],
                                    lhsT=wk_sl(db, eb),
                                    rhs=x0_ts[db],
                                    start=(db == 0),
                                    stop=(db == DB - 1),
                                )
                        with nc.allow_low_precision(
                            reason="fp8 K for DoubleRow"
                        ):
                            for eb in ebs:
                                nc.vector.tensor_copy(
                                    out=kth_t[:, eb, 0:FT], in_=pss[eb % 4]
                                )
                ka_r = ktha_d.rearrange("(db p) n -> p db n", p=P)
                nc.gpsimd.dma_start(out=ka_r[:, 0:4, :], in_=kth_t[:, 0:4, 0:FT])
                nc.gpsimd.dma_start(out=ka_r[:, 4:8, :], in_=kth_t[:, 4:8, 0:FT])
                nc.gpsimd.collective_compute(
                    "AllGather",
                    ALU.bypass,
                    replica_groups=PAIRS,
                    ins=[ktha_d[:, :]],
                    outs=[ktga_d[:, :, :]],
                )

                for eb in range(DB):
                    ps = psA.tile([P, FT], f32, tag="ps")
                    for db in range(DB):
                        nc.tensor.matmul(
                            ps,
                            lhsT=wk_sl(db, eb),
                            rhs=x1_ts[db],
                            start=(db == 0),
                            stop=(db == DB - 1),
                        )
                    with nc.allow_low_precision(reason="fp8 K for DoubleRow"):
                        nc.vector.tensor_copy(out=kth_t[:, eb, FT:NQ], in_=ps)
                kb_r = kthb_d.rearrange("(db p) n -> p db n", p=P)
                nc.gpsimd.dma_start(out=kb_r[:, 0:4, :], in_=kth_t[:, 0:4, FT:NQ])
                nc.gpsimd.dma_start(out=kb_r[:, 4:8, :], in_=kth_t[:, 4:8, FT:NQ])
                nc.gpsimd.collective_compute(
                    "AllGather",
                    ALU.bypass,
                    replica_groups=PAIRS,
                    ins=[kthb_d[:, :]],
                    outs=[ktgb_d[:, :, :]],
                )

                # V2 projection runs EARLY (right after the first K chunk) so its
                # larger collective finishes long before the AV phase
                # needs V; K tile 1 + its collective follow.
                # kthpool stays open: freeing it here would let vh_t reuse
                # kth_t's SBUF, making every V2 copy wait (WAR) on the K
                # staging DMAs.
                vhpool_ctx = tc.tile_pool(name="vhpool", bufs=1)
                vhpool = vhpool_ctx.__enter__()
                vh_t = vhpool.tile([P, KBH, D], bf16)
                x_ts = [x0_ts, x1_ts]
                for ks in range(KBH):
                    nt, kso = divmod(ks, FT // P)
                    for et in range(ET):
                        esl = slice(et * FT, (et + 1) * FT)
                        ps = psA.tile([P, FT], f32, tag="ps")
                        for db in range(DB):
                            nc.tensor.matmul(
                                ps,
                                lhsT=x_ts[nt][db][:, kso * P : (kso + 1) * P],
                                rhs=w2_t[:, db, esl],
                                start=(db == 0),
                                stop=(db == DB - 1),
                            )
                        nc.vector.tensor_copy(out=vh_t[:, ks, esl], in_=ps)
                vh_dr = vh_d.rearrange("(kb p) e -> p kb e", p=P)
                nc.gpsimd.dma_start(out=vh_dr[:, 0:4, :], in_=vh_t[:, 0:4, :])
                nc.gpsimd.dma_start(out=vh_dr[:, 4:8, :], in_=vh_t[:, 4:8, :])
                nc.gpsimd.collective_compute(
                    "AllGather",
                    ALU.bypass,
                    replica_groups=PAIRS,
                    ins=[vh_d[:, :]],
                    outs=[vg_d[:, :, :]],
                )


                # Loads issue after all collectives so the in-order gpsimd
                # queue never stalls a collective trigger behind a load.
                # Merged 3D-AP triggers (one per gather chunk).
                for nt, kg in enumerate([ktga_d, ktgb_d]):
                    for g in range(2):
                        kg_gr = kg[g].rearrange("(db p) n -> p db n", p=P)
                        off = g * NQ + nt * FT
                        nc.sync.dma_start(
                            out=KT[:, :, off : off + FT], in_=kg_gr[:, :, :]
                        )
                for g in range(2):
                    vg_gr = vg_d[g].rearrange("(kb p) e -> p kb e", p=P)
                    nc.gpsimd.dma_start(
                        out=V[:, g * KBH : (g + 1) * KBH, :],
                        in_=vg_gr[:, :, :],
                    )

                # Q projection (overlaps the collectives), written as fp8
                with nc.allow_low_precision(reason="fp8 Q for DoubleRow"):
                    for nt in range(NTH):
                        nsl = slice(nt * FT, (nt + 1) * FT)
                        for eb in range(DB):
                            ps = psA.tile([P, FT], f32, tag="ps")
                            for db in range(DB):
                                nc.tensor.matmul(
                                    ps,
                                    lhsT=wq_t[:, db, eb * P : (eb + 1) * P],
                                    rhs=x_ts[nt][db],
                                    start=(db == 0),
                                    stop=(db == DB - 1),
                                )
                            nc.scalar.activation(
                                out=QT[:, eb, nsl],
                                in_=ps,
                                func=AF.Identity,
                                bias=bq_t[:, eb : eb + 1],
                                scale=1.0,
                            )
                psA_ctx.__exit__(None, None, None)
                vhpool_ctx.__exit__(None, None, None)
                kthpool_ctx.__exit__(None, None, None)

            # ---- Phase B: scores (fp8 DoubleRow), softmax, AV, output ----
            with tc.tile_pool(name="persist2", bufs=1) as persist2:
                UT = persist2.tile([P, KB, NQ], bf16)
                sbcs = [persist2.tile([P, FT], f32, name=f"sbc{qt}")
                        for qt in range(QT_TILES)]
                s_bf = [persist2.tile([1, FT], bf16, name=f"sbf{qt}")
                        for qt in range(QT_TILES)]

                with (
                    tc.tile_pool(name="scr", bufs=3) as scr,
                    tc.tile_pool(name="psST", bufs=4, space="PSUM") as psST,
                    tc.tile_pool(name="psAV", bufs=3, space="PSUM") as psAV,
                    tc.tile_pool(name="psS", bufs=1, space="PSUM") as psS,
                    tc.tile_pool(name="ost", bufs=3) as ostp,
                ):
                    # Consume keys in K-gather arrival order:
                    # gather 0 -> kb {0-3, 8-11}, gather 1 -> kb {4-7, 12-15}
                    ST_KB_HALVES = [[0, 1, 2, 3, 8, 9, 10, 11],
                                    [4, 5, 6, 7, 12, 13, 14, 15]]

                    def st_tiles(qt, half):
                        qsl = slice(qt * FT, (qt + 1) * FT)
                        for kb in ST_KB_HALVES[half]:
                            ps = psST.tile([P, FT], f32, tag="st", name=f"ps_st{qt}_{kb}")
                            for s in range(SB):
                                nc.tensor.matmul(
                                    ps,
                                    lhsT=KT[:, 2 * s : 2 * s + 2,
                                            kb * P : (kb + 1) * P],
                                    rhs=QT[:, 2 * s : 2 * s + 2, qsl],
                                    start=(s == 0),
                                    stop=(s == SB - 1),
                                    perf_mode=DR,
                                )
                            sc = scr.tile([P, FT], f32, tag="sc", name=f"sc{qt}_{kb}")
                            nc.vector.tensor_tensor(
                                sc, ps, maskfull[:, kb, qsl], op=ALU.add
                            )
                            nc.scalar.activation(
                                out=UT[:, kb, qsl], in_=sc, func=AF.Exp,
                                scale=INV_SQRT_N,
                            )

                    def sums(qt):
                        qsl = slice(qt * FT, (qt + 1) * FT)
                        pss = psS.tile([1, FT], f32, tag="pss", name=f"pss{qt}")
                        for kb in range(KB):
                            nc.tensor.matmul(
                                pss,
                                lhsT=ones_k,
                                rhs=UT[:, kb, qsl],
                                start=(kb == 0),
                                stop=(kb == KB - 1),
                            )
                        with nc.allow_low_precision(
                            reason="softmax sums in bf16; 0.1% rms"
                        ):
                            nc.vector.tensor_copy(out=s_bf[qt], in_=pss)

                    def av_out(qt):
                        qsl = slice(qt * FT, (qt + 1) * FT)
                        psb = psAV.tile([P, FT], f32, tag="av", name=f"ps_b{qt}")
                        # bf16 K=1 broadcast of s across partitions; DVE
                        # reciprocal after, 128 partitions wide.
                        nc.tensor.matmul(
                            psb, lhsT=ones_b, rhs=s_bf[qt], start=True, stop=True
                        )
                        nc.vector.reciprocal(out=sbcs[qt], in_=psb)
                        for eb in range(DB):
                            ps = psAV.tile([P, FT], f32, tag="av", name=f"ps_av{qt}_{eb}")
                            for kb in range(KB):
                                nc.tensor.matmul(
                                    ps,
                                    lhsT=V[:, kb, eb * P : (eb + 1) * P],
                                    rhs=UT[:, kb, qsl],
                                    start=(kb == 0),
                                    stop=(kb == KB - 1),
                                )
                            if qt == QT_TILES - 1 and eb == DB - 1:
                                # Final tile: halve the epilogue chain so the
                                # first out-DMA overlaps the second multiply.
                                HF = FT // 2
                                for h in range(2):
                                    hs = slice(h * HF, (h + 1) * HF)
                                    hq = slice(qt * FT + h * HF,
                                               qt * FT + (h + 1) * HF)
                                    ot = ostp.tile([P, HF], f32, tag="oth")
                                    nc.vector.tensor_mul(
                                        ot, ps[:, hs], sbcs[qt][:, hs]
                                    )
                                    nc.sync.dma_start(
                                        out=outT[eb * P : (eb + 1) * P, hq],
                                        in_=ot,
                                    )
                            else:
                                ot = ostp.tile([P, FT], f32, tag="ot")
                                nc.vector.tensor_mul(ot, ps, sbcs[qt])
                                nc.sync.dma_start(
                                    out=outT[eb * P : (eb + 1) * P, qsl],
                                    in_=ot,
                                )

                    # Order chosen so every PE wait is covered by prior PE
                    # work: both q-tiles' gather-0 keys run before any
                    # gather-1 key is needed.
                    st_tiles(0, 0)
                    st_tiles(1, 0)
                    st_tiles(0, 1)
                    sums(0)
                    st_tiles(1, 1)
                    sums(1)
                    av_out(0)
                    av_out(1)

    nc.compile()
    return nc


def _get_program():
    global _cached
    if _cached is None:
        _cached = _build_program()
    return _cached


def make_in_maps(x, mask, Wq, bq, Wk, bk, Wv, bv, Wo, bo):
    """Host-side preprocessing: per-core input dicts."""
    wq_b = Wq.astype(BF16)
    wk_b = Wk.astype(BF16)
    w2_b = (Wv.astype(np.float64) @ Wo.astype(np.float64)).astype(BF16)
    bq_f = bq.astype(np.float32)

    in_maps = []
    for c in range(8):
        b, h = divmod(c, 2)
        qs = slice(h * NQ, (h + 1) * NQ)
        xTh_c = np.ascontiguousarray(x[b, qs].T).astype(BF16)  # [D, NQ]
        madd = np.where(
            mask[b, qs, :].T, np.float32(MASK_NEG), np.float32(0.0)
        ).astype(BF16)  # [N, NQ], global key order
        in_maps.append(
            {
                "xTh": xTh_c,
                "maskadd": np.ascontiguousarray(madd),
                "wq": wq_b,
                "wk": wk_b,
                "w2": w2_b,
                "bq": bq_f,
            }
        )
    return in_maps


def assemble(results):
    out = np.empty((B, N, D), dtype=np.float32)
    for c in range(8):
        b, h = divmod(c, 2)
        out[b, h * NQ : (h + 1) * NQ, :] = results[c]["outT"].T
    return out


def kernel(x, mask, Wq, bq, Wk, bk, Wv, bv, Wo, bo):
    from concourse.bass_utils import run_bass_kernel_spmd

    nc = _get_program()
    in_maps = make_in_maps(x, mask, Wq, bq, Wk, bk, Wv, bv, Wo, bo)
    res = run_bass_kernel_spmd(nc, in_maps, list(range(8)))
    out = assemble(res.results)
    # Attention rows sum to 1, so bo_eff = bo + bv @ Wo is a constant row
    # of the output; adding it host-side costs no device time.
    bo_eff = (
        bo.astype(np.float64) + bv.astype(np.float64) @ Wo.astype(np.float64)
    ).astype(np.float32)
    out += bo_eff
    return out
